# revision 45
# baseline (speedup 1.0000x reference)
"""Trainium2 Bass kernel for BaselineMultiStepRNN.

Math (per original reference, 1-based step index t = 1..T):
    h_t   = tanh(Wx x_t + Wc cap_{t-1} + Whh h_{t-1} + b_ih + b_hh)
    drop_t = fc_w h_t + fc_b
    cap_t = cap_{t-1} - drop_t ;  out[:, t-1] = cap_t

Folded form used on device (state v_t = cap_t - fc_b):
    W'  = Whh - outer(Wc, fc_w)     (removes cap's one-step feedback lag)
    pre_t = Wx x_t + b * 1 + Wc v_{t-2} + W' h_{t-1}
    h_t  = tanh(pre_t)
    d_t  = fc_w h_t
    v_t  = (v_{t-1} - fc_b) - d_t          (v_0 = cap_0 - fc_b, v_{-1} = cap_0)
    out[:, t-1] = v_t + fc_b

All matmuls run in native fp32 (4 cycles/row): this recurrence is mildly
chaotic (per-step perturbations amplify ~1e5x over 512 steps), so reduced
precision paths (fp32r ~12-bit mantissa) land ~400x outside the fp32
arithmetic envelope.

The tensor engine does only the 6 unavoidable full-width matmuls per step
(2 output halves x [x/bias/v K=65 + two K=128 recurrent chunks]); with
K=321 contraction rows (x 63 + ones + v + h 256) against K<=128 and
M=256 outputs against M<=128 per instruction, 6 is provably minimal, and
the steady state runs at that roofline (2562 ns/step).  The fc
projection d_t = fc_w h_t is computed OFF the tensor engine: DVE scales
each tanh half by its fc column ([128,1] per-partition scalar), GPSIMD
partition_all_reduce sums across the 128 partitions giving the per-batch
dot products directly in row form ([1, BC]), and DVE folds them into the
capacity update.  Matmuls are grouped per PSUM half (x, recK0, recK1) so
tanh of half 0 starts ~1.3us into the step and the v feedback for step
t+2's rhs row lands with slack.  Two junk matmuls plus step 2's early
x matmuls ramp the PE to full clock behind step 1 while its
tanh -> fc -> v chain (the serial seed of the pipeline) runs on the
other engines.

Layouts (per core, batch slice BC=256): everything runs transposed;
h state [128, 512] with free = kc*256 + batch (kc = hidden-row half);
PSUM pre [128, 256] per output-row half; x host-pretransposed into chunk
tiles [65, 8, BC] = [x rows(63) + ones + v-row, 8 steps, batch].
"""

import os

os.environ.setdefault("MYCRO_LOCAL_CACHE", "1")

from contextlib import ExitStack

import numpy as np

import concourse.tile as tile
from concourse import bacc, bass_isa, mybir
from concourse.alu_op_type import AluOpType
from concourse.bass_utils import run_bass_kernel_spmd

T_FULL = 512
F = 63
H = 256
B_FULL = 2048
NCORES = 8
BC = B_FULL // NCORES  # 256 batch per core
CH = 8                 # time steps per x chunk tile
F32 = mybir.dt.float32

_CACHE: dict = {}


def _build(T: int):
    if T in _CACHE:
        return _CACHE[T]

    # slot s holds step s+1's rhs rows; the last v-row any matmul reads is
    # v_{T-2} in slot T-1, so slots end at T-1 and steps T-1/T skip the
    # slot-row write entirely
    NSLOT = T
    NCHUNK = (NSLOT + CH - 1) // CH
    nc = bacc.Bacc(
        "TRN2", target_bir_lowering=False, debug=False, enable_asserts=False
    )
    xTd = nc.dram_tensor("xT", [NCHUNK, F + 2, CH, BC], F32, kind="ExternalInput").ap()
    wxbvd = nc.dram_tensor("wxbv", [F + 2, H], F32, kind="ExternalInput").ap()
    wpd = nc.dram_tensor("wp", [128, 2, H], F32, kind="ExternalInput").ap()
    fcd = nc.dram_tensor("fct", [128, 2], F32, kind="ExternalInput").ap()
    fcbd = nc.dram_tensor("fcb", [1, 1], F32, kind="ExternalInput").ap()
    vind = nc.dram_tensor("vinit", [1, BC], F32, kind="ExternalInput").ap()
    voutd = nc.dram_tensor("vout", [T, 1, BC], F32, kind="ExternalOutput").ap()

    TANH = mybir.ActivationFunctionType.Tanh
    SUB = AluOpType.subtract
    RADD = bass_isa.ReduceOp.add
    KV = F + 2  # 65 rows: x(63), ones, v

    with tile.TileContext(nc) as tc, ExitStack() as ctx:
        consts = ctx.enter_context(tc.tile_pool(name="consts", bufs=1))
        wxbv = consts.tile([KV, H], F32)
        wp = consts.tile([128, 2, H], F32)
        fct = consts.tile([128, 2], F32)
        fcb = consts.tile([1, 1], F32)
        vin = consts.tile([1, BC], F32)
        nc.sync.dma_start(wxbv[:], wxbvd[:])

        xpool = ctx.enter_context(tc.tile_pool(name="xpool", bufs=4))
        hpool = ctx.enter_context(tc.tile_pool(name="hpool", bufs=2))
        h2pool = ctx.enter_context(tc.tile_pool(name="h2pool", bufs=2))
        spool = ctx.enter_context(tc.tile_pool(name="spool", bufs=2))
        vlpool = ctx.enter_context(tc.tile_pool(name="vlpool", bufs=3))
        ppool = ctx.enter_context(tc.tile_pool(name="ppool", bufs=3, space="PSUM"))

        xtiles: dict = {}

        def xchunk(c):
            if c not in xtiles:
                xt = xpool.tile([F + 2, CH, BC], F32, name="xt", tag="xt")
                if c == 0:
                    # split so step 1's rhs (and host v_{-1}, v_0 in row F+1
                    # slots 0,1) lands ~1.2us before the rest of the chunk
                    nc.sync.dma_start(xt[:, 0:2], xTd[c][:, 0:2])
                    nc.sync.dma_start(xt[:, 2:CH], xTd[c][:, 2:CH])
                else:
                    nc.sync.dma_start(xt[0:F + 1], xTd[c, 0:F + 1])
                xtiles[c] = xt
            return xtiles[c]

        def slot_rhs(s):
            return xchunk(s // CH)[:, s % CH, :]

        def vrow(s):  # [1, BC] AP holding v_{s-1} (slot s's v row)
            return xchunk(s // CH)[F + 1:F + 2, s % CH, :]

        # step 1's rhs slots load right behind wxbv; the remaining consts
        # (needed ~2-4us later by the rec matmuls and the v chain) follow
        xchunk(0)
        nc.sync.dma_start(wp[:], wpd[:])
        nc.sync.dma_start(fct[:], fcd[:])
        nc.sync.dma_start(fcb[:], fcbd[:])
        nc.sync.dma_start(vin[:], vind[:])

        wpsum = ctx.enter_context(tc.tile_pool(name="wpsum", bufs=1, space="PSUM"))
        warm = wpsum.tile([128, BC], F32, name="warm", tag="warm")
        wsrc = consts.tile([128, BC], F32)
        nc.vector.memset(wsrc[:], 0.0)

        h_prev = None
        vprev = None
        hp_next = None

        for t in range(1, T + 1):
            first = h_prev is None
            rx = slot_rhs(t - 1)
            # per output-row half: x/bias/v matmul opens the PSUM group,
            # then the two recurrent K chunks; half 0 closes 3 matmuls in
            # so its tanh (and the whole fc/v chain) starts early.
            if hp_next is None:
                hp = [
                    ppool.tile([128, BC], F32, name="hp0", tag="hp0"),
                    ppool.tile([128, BC], F32, name="hp1", tag="hp1"),
                ]
                for mt in range(2):
                    nc.tensor.matmul(
                        hp[mt][:], wxbv[:, mt * 128:(mt + 1) * 128], rx,
                        start=True, stop=first,
                    )
            else:
                hp = hp_next            # x matmuls already issued at t-1
                hp_next = None
            if not first:
                for mt in range(2):
                    for kc in range(2):
                        nc.tensor.matmul(
                            hp[mt][:],
                            wp[:, kc, mt * 128:(mt + 1) * 128],
                            h_prev[:, kc * BC:(kc + 1) * BC],
                            start=False, stop=(kc == 1),
                        )
            if first and T >= 2:
                # ramp the PE to full clock (takes ~3us of continuous busy)
                # behind step 1's matmuls, while its tanh -> fc -> v chain
                # (the serial seed of the pipeline) runs on the other
                # engines; step 2's x matmuls (whose v-row is host-seeded)
                # double as ramp filler
                nc.tensor.matmul(
                    warm[:], wsrc[:, 0:128], wsrc[:], start=True, stop=True
                )
                hp_next = [
                    ppool.tile([128, BC], F32, name="hp0", tag="hp0"),
                    ppool.tile([128, BC], F32, name="hp1", tag="hp1"),
                ]
                rx2 = slot_rhs(1)
                for mt in range(2):
                    nc.tensor.matmul(
                        hp_next[mt][:], wxbv[:, mt * 128:(mt + 1) * 128], rx2,
                        start=True, stop=False,
                    )
                nc.tensor.matmul(
                    warm[:], wsrc[:, 0:128], wsrc[:], start=True, stop=True
                )
            h = hpool.tile([128, 2 * BC], F32, name="h", tag="h")
            h2a = h2pool.tile([128, BC], F32, name="h2a", tag="h2a")
            h2b = h2pool.tile([128, BC], F32, name="h2b", tag="h2b")
            s = spool.tile([128, BC], F32, name="s", tag="s")
            # tanh halves; DVE folds both fc-scaled halves into one tensor so
            # GPSIMD does a single partition-sum -> d_t row ([1, BC]).
            nc.scalar.activation(h[:, 0:BC], hp[0][:], TANH)
            nc.vector.tensor_scalar_mul(h2a[:], h[:, 0:BC], fct[:, 0:1])
            nc.scalar.activation(h[:, BC:2 * BC], hp[1][:], TANH)
            nc.vector.scalar_tensor_tensor(
                h2b[:], h[:, BC:2 * BC], fct[:, 1:2], h2a[:],
                op0=AluOpType.mult, op1=AluOpType.add,
            )
            nc.gpsimd.partition_all_reduce(
                s[:], h2b[:], channels=128, reduce_op=RADD
            )
            # v_t = (v_{t-1} - fcb) - d_t, into slot t+1's rhs row and into a
            # partition-0 chain tile (DVE SBUF inputs must share a partition
            # base, so the slot row is write-only).
            prev = vin[:] if vprev is None else vprev[:]
            if t <= T - 2:
                nc.vector.scalar_tensor_tensor(
                    vrow(t + 1), prev, fcb[0:1, 0:1], s[0:1, :], op0=SUB, op1=SUB
                )
            v = vlpool.tile([1, BC], F32, name="v", tag="v")
            nc.vector.scalar_tensor_tensor(
                v[:], prev, fcb[0:1, 0:1], s[0:1, :], op0=SUB, op1=SUB
            )
            nc.sync.dma_start(voutd[t - 1], v[:])
            vprev = v
            h_prev = h

    nc.compile()
    _CACHE[T] = nc
    return nc


def _prep_maps(x_seq, seed_capacity, W_ih_w, W_ih_b, W_hh_w, W_hh_b, fc_w, fc_b, T):
    x_seq = np.asarray(x_seq, dtype=np.float32)
    seed = np.asarray(seed_capacity, dtype=np.float32).reshape(B_FULL)
    W_ih_w = np.asarray(W_ih_w, dtype=np.float32)
    W_ih_b = np.asarray(W_ih_b, dtype=np.float32)
    W_hh_w = np.asarray(W_hh_w, dtype=np.float32)
    W_hh_b = np.asarray(W_hh_b, dtype=np.float32)
    fc_w = np.asarray(fc_w, dtype=np.float32)
    fc_b = np.asarray(fc_b, dtype=np.float32)

    Wx = W_ih_w[:, :F]            # [H, 63]
    Wc = W_ih_w[:, F]             # [H]
    bvec = W_ih_b + W_hh_b        # [H]
    fcb_val = float(fc_b[0])

    wxbv = np.concatenate(
        [Wx.T, bvec[None, :], Wc[None, :]], axis=0
    ).astype(np.float32)                                         # [65, H]
    Wp = W_hh_w - np.outer(Wc, fc_w[0])
    wp = np.ascontiguousarray(Wp.T.reshape(2, 128, H).transpose(1, 0, 2))
    fct = np.ascontiguousarray(fc_w[0].reshape(2, 128).T)        # [128, 2]
    fcb = np.array([[fcb_val]], dtype=np.float32)

    NSLOT = T
    NCHUNK = (NSLOT + CH - 1) // CH

    in_maps = []
    for c in range(NCORES):
        sl = slice(c * BC, (c + 1) * BC)
        xc = x_seq[sl, :T, :]                                    # [BC, T, F]
        xtr = np.ascontiguousarray(xc.transpose(1, 2, 0))        # [T, F, BC]
        Tp = NCHUNK * CH
        xtr = np.concatenate(
            [xtr, np.zeros((Tp - T, F, BC), np.float32)], axis=0
        )
        xT = np.zeros((NCHUNK, F + 2, CH, BC), np.float32)
        xT[:, :F] = xtr.reshape(NCHUNK, CH, F, BC).transpose(0, 2, 1, 3)
        xT[:, F] = 1.0                                            # ones row
        seedc = seed[sl]                                          # cap_0
        v0 = (seedc - fcb_val).astype(np.float32)
        xT[0, F + 1, 0] = seedc                                   # v_{-1}
        if T >= 2:
            xT[0, F + 1, 1] = v0                                  # v_0
        in_maps.append(
            {
                "xT": np.ascontiguousarray(xT),
                "wxbv": wxbv,
                "wp": wp,
                "fct": fct,
                "fcb": fcb,
                "vinit": v0[None, :].copy(),
            }
        )
    return in_maps, fcb_val


def _run(trace=False, **inputs):
    T = int(inputs.get("forecast_steps", T_FULL))
    nc = _build(T)
    in_maps, fcb_val = _prep_maps(
        inputs["x_seq"], inputs["seed_capacity"],
        inputs["W_ih_w"], inputs["W_ih_b"],
        inputs["W_hh_w"], inputs["W_hh_b"],
        inputs["fc_w"], inputs["fc_b"], T,
    )
    res = run_bass_kernel_spmd(
        nc, in_maps, core_ids=list(range(NCORES)), trace=trace
    )
    out = np.empty((B_FULL, T), np.float32)
    for c in range(NCORES):
        v = res.results[c]["vout"].reshape(T, BC)
        out[c * BC:(c + 1) * BC] = (v + fcb_val).T
    return out, res


def kernel(**inputs) -> np.ndarray:
    out, _ = _run(trace=False, **inputs)
    return out


# revision 46
# speedup vs baseline: 1.0003x; 1.0003x over previous
"""Trainium2 Bass kernel for BaselineMultiStepRNN.

Math (per original reference, 1-based step index t = 1..T):
    h_t   = tanh(Wx x_t + Wc cap_{t-1} + Whh h_{t-1} + b_ih + b_hh)
    drop_t = fc_w h_t + fc_b
    cap_t = cap_{t-1} - drop_t ;  out[:, t-1] = cap_t

Folded form used on device (state v_t = cap_t - fc_b):
    W'  = Whh - outer(Wc, fc_w)     (removes cap's one-step feedback lag)
    pre_t = Wx x_t + b * 1 + Wc v_{t-2} + W' h_{t-1}
    h_t  = tanh(pre_t)
    d_t  = fc_w h_t
    v_t  = (v_{t-1} - fc_b) - d_t          (v_0 = cap_0 - fc_b, v_{-1} = cap_0)
    out[:, t-1] = v_t + fc_b

All matmuls run in native fp32 (4 cycles/row): this recurrence is mildly
chaotic (per-step perturbations amplify ~1e5x over 512 steps), so reduced
precision paths (fp32r ~12-bit mantissa) land ~400x outside the fp32
arithmetic envelope.

The tensor engine does only the 6 unavoidable full-width matmuls per step
(2 output halves x [x/bias/v K=65 + two K=128 recurrent chunks]); with
K=321 contraction rows (x 63 + ones + v + h 256) against K<=128 and
M=256 outputs against M<=128 per instruction, 6 is provably minimal, and
the steady state runs at that roofline (2562 ns/step).  The fc
projection d_t = fc_w h_t is computed OFF the tensor engine: DVE scales
each tanh half by its fc column ([128,1] per-partition scalar), GPSIMD
partition_all_reduce sums across the 128 partitions giving the per-batch
dot products directly in row form ([1, BC]), and DVE folds them into the
capacity update.  Matmuls are grouped per PSUM half (x, recK0, recK1) so
tanh of half 0 starts ~1.3us into the step and the v feedback for step
t+2's rhs row lands with slack.  Two junk matmuls plus step 2's early
x matmuls ramp the PE to full clock behind step 1 while its
tanh -> fc -> v chain (the serial seed of the pipeline) runs on the
other engines.

Layouts (per core, batch slice BC=256): everything runs transposed;
h state [128, 512] with free = kc*256 + batch (kc = hidden-row half);
PSUM pre [128, 256] per output-row half; x host-pretransposed into chunk
tiles [65, 8, BC] = [x rows(63) + ones + v-row, 8 steps, batch].
"""

import os

os.environ.setdefault("MYCRO_LOCAL_CACHE", "1")

from contextlib import ExitStack

import numpy as np

import concourse.tile as tile
from concourse import bacc, bass_isa, mybir
from concourse.alu_op_type import AluOpType
from concourse.bass_utils import run_bass_kernel_spmd

T_FULL = 512
F = 63
H = 256
B_FULL = 2048
NCORES = 8
BC = B_FULL // NCORES  # 256 batch per core
CH = 8                 # time steps per x chunk tile
F32 = mybir.dt.float32

_CACHE: dict = {}


def _build(T: int):
    if T in _CACHE:
        return _CACHE[T]

    # slot s holds step s+1's rhs rows; the last v-row any matmul reads is
    # v_{T-2} in slot T-1, so slots end at T-1 and steps T-1/T skip the
    # slot-row write entirely
    NSLOT = T
    NCHUNK = (NSLOT + CH - 1) // CH
    nc = bacc.Bacc(
        "TRN2", target_bir_lowering=False, debug=False, enable_asserts=False
    )
    xTd = nc.dram_tensor("xT", [NCHUNK, F + 2, CH, BC], F32, kind="ExternalInput").ap()
    wxbvd = nc.dram_tensor("wxbv", [F + 2, H], F32, kind="ExternalInput").ap()
    wpd = nc.dram_tensor("wp", [128, 2, H], F32, kind="ExternalInput").ap()
    fcd = nc.dram_tensor("fct", [128, 2], F32, kind="ExternalInput").ap()
    fcbd = nc.dram_tensor("fcb", [1, 1], F32, kind="ExternalInput").ap()
    vind = nc.dram_tensor("vinit", [1, BC], F32, kind="ExternalInput").ap()
    voutd = nc.dram_tensor("vout", [T, 1, BC], F32, kind="ExternalOutput").ap()

    TANH = mybir.ActivationFunctionType.Tanh
    SUB = AluOpType.subtract
    RADD = bass_isa.ReduceOp.add
    KV = F + 2  # 65 rows: x(63), ones, v

    with tile.TileContext(nc) as tc, ExitStack() as ctx:
        consts = ctx.enter_context(tc.tile_pool(name="consts", bufs=1))
        wxbv = consts.tile([KV, H], F32)
        wp = consts.tile([128, 2, H], F32)
        fct = consts.tile([128, 2], F32)
        fcb = consts.tile([1, 1], F32)
        vin = consts.tile([1, BC], F32)
        nc.sync.dma_start(wxbv[:], wxbvd[:])

        xpool = ctx.enter_context(tc.tile_pool(name="xpool", bufs=4))
        hpool = ctx.enter_context(tc.tile_pool(name="hpool", bufs=2))
        h2pool = ctx.enter_context(tc.tile_pool(name="h2pool", bufs=2))
        spool = ctx.enter_context(tc.tile_pool(name="spool", bufs=2))
        vlpool = ctx.enter_context(tc.tile_pool(name="vlpool", bufs=3))
        ppool = ctx.enter_context(tc.tile_pool(name="ppool", bufs=3, space="PSUM"))

        xtiles: dict = {}

        def xchunk(c):
            if c not in xtiles:
                xt = xpool.tile([F + 2, CH, BC], F32, name="xt", tag="xt")
                if c == 0:
                    # split so step 1's rhs (and host v_{-1}, v_0 in row F+1
                    # slots 0,1) lands ~1.2us before the rest of the chunk
                    nc.sync.dma_start(xt[:, 0:2], xTd[c][:, 0:2])
                    nc.sync.dma_start(xt[:, 2:CH], xTd[c][:, 2:CH])
                else:
                    nc.sync.dma_start(xt[0:F + 1], xTd[c, 0:F + 1])
                xtiles[c] = xt
            return xtiles[c]

        def slot_rhs(s):
            return xchunk(s // CH)[:, s % CH, :]

        def vrow(s):  # [1, BC] AP holding v_{s-1} (slot s's v row)
            return xchunk(s // CH)[F + 1:F + 2, s % CH, :]

        # step 1's rhs slots load right behind wxbv; the remaining consts
        # (needed ~2-4us later by the rec matmuls and the v chain) follow
        xchunk(0)
        nc.sync.dma_start(wp[:], wpd[:])
        nc.sync.dma_start(fct[:], fcd[:])
        nc.sync.dma_start(fcb[:], fcbd[:])
        nc.sync.dma_start(vin[:], vind[:])

        wpsum = ctx.enter_context(tc.tile_pool(name="wpsum", bufs=1, space="PSUM"))
        warm = wpsum.tile([128, BC], F32, name="warm", tag="warm")
        wsrc = consts.tile([128, BC], F32)
        nc.vector.memset(wsrc[:], 0.0)

        h_prev = None
        vprev = None
        hp_next = None

        for t in range(1, T + 1):
            first = h_prev is None
            rx = slot_rhs(t - 1)
            # per output-row half: x/bias/v matmul opens the PSUM group,
            # then the two recurrent K chunks; half 0 closes 3 matmuls in
            # so its tanh (and the whole fc/v chain) starts early.
            if hp_next is None:
                hp = [
                    ppool.tile([128, BC], F32, name="hp0", tag="hp0"),
                    ppool.tile([128, BC], F32, name="hp1", tag="hp1"),
                ]
                for mt in range(2):
                    nc.tensor.matmul(
                        hp[mt][:], wxbv[:, mt * 128:(mt + 1) * 128], rx,
                        start=True, stop=first,
                    )
            else:
                hp = hp_next            # x matmuls already issued at t-1
                hp_next = None
            if not first:
                for mt in range(2):
                    for kc in range(2):
                        nc.tensor.matmul(
                            hp[mt][:],
                            wp[:, kc, mt * 128:(mt + 1) * 128],
                            h_prev[:, kc * BC:(kc + 1) * BC],
                            start=False, stop=(kc == 1),
                        )
            if first and T >= 2:
                # ramp the PE to full clock (takes ~3us of continuous busy)
                # behind step 1's matmuls, while its tanh -> fc -> v chain
                # (the serial seed of the pipeline) runs on the other
                # engines; step 2's x matmuls (whose v-row is host-seeded)
                # double as ramp filler
                nc.tensor.matmul(
                    warm[:], wsrc[:, 0:128], wsrc[:], start=True, stop=True
                )
                hp_next = [
                    ppool.tile([128, BC], F32, name="hp0", tag="hp0"),
                    ppool.tile([128, BC], F32, name="hp1", tag="hp1"),
                ]
                rx2 = slot_rhs(1)
                for mt in range(2):
                    nc.tensor.matmul(
                        hp_next[mt][:], wxbv[:, mt * 128:(mt + 1) * 128], rx2,
                        start=True, stop=False,
                    )
                nc.tensor.matmul(
                    warm[:], wsrc[:, 0:128], wsrc[:], start=True, stop=True
                )
            h = hpool.tile([128, 2 * BC], F32, name="h", tag="h")
            nc.scalar.activation(h[:, 0:BC], hp[0][:], TANH)
            prev = vin[:] if vprev is None else vprev[:]
            v = vlpool.tile([1, BC], F32, name="v", tag="v")
            if t < T:
                # tanh halves; DVE folds both fc-scaled halves into one
                # tensor so GPSIMD does a single partition-sum -> d_t row.
                h2a = h2pool.tile([128, BC], F32, name="h2a", tag="h2a")
                h2b = h2pool.tile([128, BC], F32, name="h2b", tag="h2b")
                s = spool.tile([128, BC], F32, name="s", tag="s")
                nc.vector.tensor_scalar_mul(h2a[:], h[:, 0:BC], fct[:, 0:1])
                nc.scalar.activation(h[:, BC:2 * BC], hp[1][:], TANH)
                nc.vector.scalar_tensor_tensor(
                    h2b[:], h[:, BC:2 * BC], fct[:, 1:2], h2a[:],
                    op0=AluOpType.mult, op1=AluOpType.add,
                )
                nc.gpsimd.partition_all_reduce(
                    s[:], h2b[:], channels=128, reduce_op=RADD
                )
                d = s[0:1, :]
            else:
                # final step: the tensor engine is drained, so the row-form
                # fc matmul pair beats the DVE->GPSIMD chain latency there
                nc.scalar.activation(h[:, BC:2 * BC], hp[1][:], TANH)
                dp = wpsum.tile([1, BC], F32, name="dlast", tag="dlast")
                nc.tensor.matmul(dp[:], fct[:, 0:1], h[:, 0:BC],
                                 start=True, stop=False)
                nc.tensor.matmul(dp[:], fct[:, 1:2], h[:, BC:2 * BC],
                                 start=False, stop=True)
                d = dp[0:1, :]
            # v_t = (v_{t-1} - fcb) - d_t, into slot t+1's rhs row and into a
            # partition-0 chain tile (DVE SBUF inputs must share a partition
            # base, so the slot row is write-only).
            if t <= T - 2:
                nc.vector.scalar_tensor_tensor(
                    vrow(t + 1), prev, fcb[0:1, 0:1], d, op0=SUB, op1=SUB
                )
            nc.vector.scalar_tensor_tensor(
                v[:], prev, fcb[0:1, 0:1], d, op0=SUB, op1=SUB
            )
            nc.sync.dma_start(voutd[t - 1], v[:])
            vprev = v
            h_prev = h

    nc.compile()
    _CACHE[T] = nc
    return nc


def _prep_maps(x_seq, seed_capacity, W_ih_w, W_ih_b, W_hh_w, W_hh_b, fc_w, fc_b, T):
    x_seq = np.asarray(x_seq, dtype=np.float32)
    seed = np.asarray(seed_capacity, dtype=np.float32).reshape(B_FULL)
    W_ih_w = np.asarray(W_ih_w, dtype=np.float32)
    W_ih_b = np.asarray(W_ih_b, dtype=np.float32)
    W_hh_w = np.asarray(W_hh_w, dtype=np.float32)
    W_hh_b = np.asarray(W_hh_b, dtype=np.float32)
    fc_w = np.asarray(fc_w, dtype=np.float32)
    fc_b = np.asarray(fc_b, dtype=np.float32)

    Wx = W_ih_w[:, :F]            # [H, 63]
    Wc = W_ih_w[:, F]             # [H]
    bvec = W_ih_b + W_hh_b        # [H]
    fcb_val = float(fc_b[0])

    wxbv = np.concatenate(
        [Wx.T, bvec[None, :], Wc[None, :]], axis=0
    ).astype(np.float32)                                         # [65, H]
    Wp = W_hh_w - np.outer(Wc, fc_w[0])
    wp = np.ascontiguousarray(Wp.T.reshape(2, 128, H).transpose(1, 0, 2))
    fct = np.ascontiguousarray(fc_w[0].reshape(2, 128).T)        # [128, 2]
    fcb = np.array([[fcb_val]], dtype=np.float32)

    NSLOT = T
    NCHUNK = (NSLOT + CH - 1) // CH

    in_maps = []
    for c in range(NCORES):
        sl = slice(c * BC, (c + 1) * BC)
        xc = x_seq[sl, :T, :]                                    # [BC, T, F]
        xtr = np.ascontiguousarray(xc.transpose(1, 2, 0))        # [T, F, BC]
        Tp = NCHUNK * CH
        xtr = np.concatenate(
            [xtr, np.zeros((Tp - T, F, BC), np.float32)], axis=0
        )
        xT = np.zeros((NCHUNK, F + 2, CH, BC), np.float32)
        xT[:, :F] = xtr.reshape(NCHUNK, CH, F, BC).transpose(0, 2, 1, 3)
        xT[:, F] = 1.0                                            # ones row
        seedc = seed[sl]                                          # cap_0
        v0 = (seedc - fcb_val).astype(np.float32)
        xT[0, F + 1, 0] = seedc                                   # v_{-1}
        if T >= 2:
            xT[0, F + 1, 1] = v0                                  # v_0
        in_maps.append(
            {
                "xT": np.ascontiguousarray(xT),
                "wxbv": wxbv,
                "wp": wp,
                "fct": fct,
                "fcb": fcb,
                "vinit": v0[None, :].copy(),
            }
        )
    return in_maps, fcb_val


def _run(trace=False, **inputs):
    T = int(inputs.get("forecast_steps", T_FULL))
    nc = _build(T)
    in_maps, fcb_val = _prep_maps(
        inputs["x_seq"], inputs["seed_capacity"],
        inputs["W_ih_w"], inputs["W_ih_b"],
        inputs["W_hh_w"], inputs["W_hh_b"],
        inputs["fc_w"], inputs["fc_b"], T,
    )
    res = run_bass_kernel_spmd(
        nc, in_maps, core_ids=list(range(NCORES)), trace=trace
    )
    out = np.empty((B_FULL, T), np.float32)
    for c in range(NCORES):
        v = res.results[c]["vout"].reshape(T, BC)
        out[c * BC:(c + 1) * BC] = (v + fcb_val).T
    return out, res


def kernel(**inputs) -> np.ndarray:
    out, _ = _run(trace=False, **inputs)
    return out
